# revision 10
# baseline (speedup 1.0000x reference)
"""GCN forward (2x graph-conv + global max-pool + linear) on 8 TRN2 NeuronCores.

Reference computation (N=16384 nodes, 256 feats, 64 hid):
    h1 = relu(adj @ (x @ W1) + b1)          [N, 64]
    h2 = adj @ (h1 @ W2) + b2               [N, 2]
    out = max(h2, axis=0) @ W3.T + b3       [1, 1, 1]

Distribution: row-shard adj over the 8 cores (core c owns output rows
[c*2048, (c+1)*2048)).  Each core:
  stage 1: computes the full xW1 = x @ W1 [N, 64] (replicated, tiny)
  pass A : h1T = (xW1.T @ adjT_shard)      [64, 2048]  (streams its 64MiB
           bf16 adjT shard from HBM, contraction over all N nodes)
           + bias/relu fused on evacuation
  stage 3: g_local = h1 @ W2               [2048, 2]
  AllGather g_local -> g_full [N, 2] (on-device collective, ~64KB)
  pass B : h2T = (g_full.T @ adjT_shard)   [2, 2048]   (re-streams the shard)
  local max over free axis -> [2, 1] per-core output
Host: max over cores, + b2, @ W3.T + b3 (b2/W3/b3 kept exactly in f32).

All matmul operands are bf16 (fp32 PSUM accumulation).  bf16 is safe here:
output is dominated by the f32 bias terms, and the adjacency's positive mean
causes quantization noise in the big contractions to average down by ~sqrt(N).
Measured end-to-end rel-err vs the f32 reference is ~1e-5.

The kernel is DMA-bound: 2 passes x 64MiB bf16 adj per core ~= 128MiB at
~360GB/s -> ~360us floor; PE work is ~240us and hides under the DMA stream.
"""

import os
import sys

sys.path.insert(0, "/opt/trn_rl_repo")

import numpy as np
import ml_dtypes


def _install_ntff_hook_shim():
    """The image's `antenv` lacks `axon_hooks`, which bass_utils imports for
    trace=True under axon. Provide it, wired to the PJRT .so's NRT-profile
    C ABI (same thing trn_boot would have registered)."""
    import types
    if "antenv.axon_hooks" in sys.modules:
        return
    try:
        import antenv  # noqa: F401
        from trn_agent_boot.trn_boot import _ntff_profile_via_ctypes
        mod = types.ModuleType("antenv.axon_hooks")
        _state = {"hook": _ntff_profile_via_ctypes("/opt/axon/libaxon_pjrt.so")}
        mod.set_axon_ntff_profile_hook = lambda h: _state.update(hook=h)
        mod.get_axon_ntff_profile_hook = lambda: _state["hook"]
        sys.modules["antenv.axon_hooks"] = mod
    except Exception:
        pass


_install_ntff_hook_shim()

import concourse.bass as bass
import concourse.mybir as mybir
import concourse.tile as tile
from concourse import bacc
from concourse.bass_utils import run_bass_kernel_spmd

BF16_NP = ml_dtypes.bfloat16

P = 128          # partition dim
N_CORES = 8
N_NODES = 16384
N_FEAT = 256
N_HID = 64


class Cfg:
    def __init__(self, n=N_NODES, n_feat=N_FEAT, n_hid=N_HID, n_cores=N_CORES,
                 iw=512, kpg=16, mpg=8, adj_bufs=4, xt_bufs=2):
        self.n, self.n_feat, self.n_hid, self.n_cores = n, n_feat, n_hid, n_cores
        self.rows = n // n_cores       # output rows per core
        self.iw = iw                   # i-tile width (psum free dim)
        self.kpg = kpg                 # k-chunks (128 nodes each) per adj DMA
        self.mpg = mpg                 # m-chunks per xt DMA
        self.kc = n // P               # contraction chunks (over all nodes)
        self.nkg = self.kc // kpg      # adj DMA groups per i-chunk
        self.ni = self.rows // iw      # i-chunks per core
        self.mc = n // P               # stage-1 m-chunks (all nodes)
        self.nmg = self.mc // mpg      # xt DMA groups
        self.mcl = self.rows // P      # stage-3 m-chunks (local rows)
        self.fkc = n_feat // P         # feature contraction chunks
        self.adj_bufs = adj_bufs
        self.xt_bufs = xt_bufs
        assert self.rows % iw == 0 and self.kc % kpg == 0 and self.mc % mpg == 0
        assert self.iw % P == 0


def build_nc(cfg: Cfg) -> bass.Bass:
    BF = mybir.dt.bfloat16
    F32 = mybir.dt.float32
    n_hid, iw, kpg, fkc = cfg.n_hid, cfg.iw, cfg.kpg, cfg.fkc

    nc = bacc.Bacc("TRN2", target_bir_lowering=False)
    # adjt[n_i, kg][p, kl*iw + ii] = adjT_shard[128*(kg*kpg+kl)+p, iw*n_i+ii]
    adjt_h = nc.declare_dram_parameter(
        "adjt", [cfg.ni, cfg.nkg, P, kpg * iw], BF, isOutput=False)
    # xt[mg][p, (ml*fkc+k)*128 + c] = x[128*(mg*mpg+ml)+c, 128*k+p]
    xt_h = nc.declare_dram_parameter(
        "xt", [cfg.nmg, P, cfg.mpg * fkc * P], BF, isOutput=False)
    w1_h = nc.declare_dram_parameter("w1", [fkc, P, n_hid], BF, isOutput=False)
    b1_h = nc.declare_dram_parameter("b1", [n_hid, 1], F32, isOutput=False)
    w2_h = nc.declare_dram_parameter("w2", [n_hid, 2], BF, isOutput=False)
    out_h = nc.declare_dram_parameter("out", [2, 1], F32, isOutput=True)

    # collective bounce buffers: g_in[p, 2*m+t] = g_local[128*m+p, t]
    g_in = nc.dram_tensor("g_in", [P, 2 * cfg.mcl], F32)
    g_out = nc.dram_tensor(
        "g_out", [P * cfg.n_cores, 2 * cfg.mcl], F32, addr_space="Shared")

    with tile.TileContext(nc, num_cores=cfg.n_cores) as tc:
        with (
            tc.tile_pool(name="const", bufs=1) as const_pool,
            tc.tile_pool(name="xw1p", bufs=1) as xw1_pool,
            tc.tile_pool(name="h1tp", bufs=1) as h1t_pool,
            tc.tile_pool(name="xtp", bufs=cfg.xt_bufs) as xt_pool,
            tc.tile_pool(name="adjp", bufs=cfg.adj_bufs) as adj_pool,
            tc.tile_pool(name="gp", bufs=1) as g_pool,
            tc.tile_pool(name="mxp", bufs=1) as mx_pool,
            tc.tile_pool(name="ps1p", bufs=2, space="PSUM") as ps1_pool,
            tc.tile_pool(name="psAp", bufs=2, space="PSUM") as psA_pool,
            tc.tile_pool(name="ps3p", bufs=2, space="PSUM") as ps3_pool,
            tc.tile_pool(name="psBp", bufs=2, space="PSUM") as psB_pool,
        ):
            # ---- constants to SBUF
            w1_sb = const_pool.tile([P, fkc * n_hid], BF)
            for k in range(fkc):
                nc.gpsimd.dma_start(
                    out=w1_sb[:, k * n_hid:(k + 1) * n_hid], in_=w1_h[k])
            b1_sb = const_pool.tile([n_hid, 1], F32)
            nc.gpsimd.dma_start(out=b1_sb[:, :], in_=b1_h[:, :])
            w2_sb = const_pool.tile([n_hid, 2], BF)
            nc.gpsimd.dma_start(out=w2_sb[:, :], in_=w2_h[:, :])

            # ---- stage 1: xW1 [n, n_hid] node-major bf16, resident in SBUF
            xw1_sb = xw1_pool.tile([P, cfg.mc * n_hid], BF)
            for mg in range(cfg.nmg):
                xt_t = xt_pool.tile([P, cfg.mpg * fkc * P], BF, tag="xt")
                nc.gpsimd.dma_start(out=xt_t[:, :], in_=xt_h[mg])
                for ml in range(cfg.mpg):
                    m = mg * cfg.mpg + ml
                    ps1 = ps1_pool.tile([P, n_hid], F32, tag="ps1")
                    for k in range(fkc):
                        nc.tensor.matmul(
                            ps1[:, :],
                            lhsT=xt_t[:, (ml * fkc + k) * P:(ml * fkc + k + 1) * P],
                            rhs=w1_sb[:, k * n_hid:(k + 1) * n_hid],
                            start=(k == 0), stop=(k == fkc - 1),
                        )
                    nc.vector.tensor_copy(
                        xw1_sb[:, m * n_hid:(m + 1) * n_hid], ps1[:, :])

            # ---- pass A: h1T = xW1.T @ adjT_shard, + fused bias/relu
            # ---- stage 3: g_local = h1 @ W2 (emitted per i-chunk)
            h1t_sb = h1t_pool.tile([n_hid, cfg.rows], BF)
            gl_sb = g_pool.tile([P, 2 * cfg.mcl], F32)
            for n_i in range(cfg.ni):
                psA = psA_pool.tile([n_hid, iw], F32, tag="psA")
                for kg in range(cfg.nkg):
                    at = adj_pool.tile([P, kpg * iw], BF, tag="at")
                    nc.gpsimd.dma_start(out=at[:, :], in_=adjt_h[n_i, kg])
                    for kl in range(kpg):
                        k = kg * kpg + kl
                        nc.tensor.matmul(
                            psA[:, :],
                            lhsT=xw1_sb[:, k * n_hid:(k + 1) * n_hid],
                            rhs=at[:, kl * iw:(kl + 1) * iw],
                            start=(k == 0), stop=(k == cfg.kc - 1),
                        )
                nc.scalar.activation(
                    h1t_sb[:, n_i * iw:(n_i + 1) * iw], psA[:, :],
                    mybir.ActivationFunctionType.Relu,
                    bias=b1_sb[:, :], scale=1.0,
                )
                for ml in range(iw // P):
                    m = n_i * (iw // P) + ml
                    ps3 = ps3_pool.tile([P, 2], F32, tag="ps3")
                    nc.tensor.matmul(
                        ps3[:, :],
                        lhsT=h1t_sb[:, m * P:(m + 1) * P],
                        rhs=w2_sb[:, :],
                        start=True, stop=True,
                    )
                    nc.vector.tensor_copy(gl_sb[:, 2 * m:2 * m + 2], ps3[:, :])
            nc.gpsimd.dma_start(out=g_in[:, :], in_=gl_sb[:, :])

            # ---- AllGather g across the 8 cores (via HBM bounce buffers)
            nc.gpsimd.collective_compute(
                "AllGather", mybir.AluOpType.bypass,
                ins=[g_in[:, :]], outs=[g_out[:, :]],
                replica_groups=[list(range(cfg.n_cores))],
            )
            # g_out[(r*128+p), 2*m+t] -> node-major g_sb[p, 2*(r*mcl+m)+t]
            gf_sb = g_pool.tile([P, 2 * cfg.kc], F32)
            nc.gpsimd.dma_start(
                out=gf_sb[:, :].rearrange("p (r c) -> p r c", r=cfg.n_cores),
                in_=g_out[:, :].rearrange("(r p) c -> p r c", p=P))
            g_sb = g_pool.tile([P, 2 * cfg.kc], BF)
            nc.vector.tensor_copy(g_sb[:, :], gf_sb[:, :])

            # ---- pass B: h2T = g_full.T @ adjT_shard; running max over i
            mxall = mx_pool.tile([2, max(cfg.ni, 2)], F32)
            for n_i in range(cfg.ni):
                psB = psB_pool.tile([2, iw], F32, tag="psB")
                for kg in range(cfg.nkg):
                    at = adj_pool.tile([P, kpg * iw], BF, tag="at")
                    nc.gpsimd.dma_start(out=at[:, :], in_=adjt_h[n_i, kg])
                    for kl in range(kpg):
                        k = kg * kpg + kl
                        nc.tensor.matmul(
                            psB[:, :],
                            lhsT=g_sb[:, 2 * k:2 * (k + 1)],
                            rhs=at[:, kl * iw:(kl + 1) * iw],
                            start=(k == 0), stop=(k == cfg.kc - 1),
                        )
                nc.vector.reduce_max(
                    mxall[:, n_i:n_i + 1], psB[:, :], axis=mybir.AxisListType.X)
            mx = mx_pool.tile([2, 1], F32)
            nc.vector.reduce_max(
                mx[:, :], mxall[:, :cfg.ni], axis=mybir.AxisListType.X)
            nc.gpsimd.dma_start(out=out_h[:, :], in_=mx[:, :])
    nc.compile()
    return nc


def shard_inputs(cfg: Cfg, x, adj, W1, b1, W2):
    """Host-side prep: cast to bf16 and pre-tile so every big DMA is a fully
    contiguous read with the partition dim laid out for direct SBUF landing."""
    x = np.asarray(x, dtype=np.float32)
    adj = np.asarray(adj, dtype=np.float32)

    # xt[mg, p, ml, k, c] = x[128*(mg*mpg+ml)+c, 128*k+p]
    xb = x.astype(BF16_NP)
    xt = xb.reshape(cfg.nmg, cfg.mpg, P, cfg.fkc, P).transpose(0, 4, 1, 3, 2)
    xt = np.ascontiguousarray(xt).reshape(cfg.nmg, P, cfg.mpg * cfg.fkc * P)

    w1 = np.ascontiguousarray(
        np.asarray(W1, dtype=np.float32).astype(BF16_NP).reshape(cfg.fkc, P, cfg.n_hid))
    b1d = np.ascontiguousarray(
        np.asarray(b1, dtype=np.float32).reshape(cfg.n_hid, 1))
    w2 = np.ascontiguousarray(np.asarray(W2, dtype=np.float32).astype(BF16_NP))

    in_maps = []
    for c in range(cfg.n_cores):
        shard = adj[c * cfg.rows:(c + 1) * cfg.rows, :].astype(BF16_NP)
        # a[n_i, kg, p, kl, ii] = shard[iw*n_i+ii, 128*(kg*kpg+kl)+p]
        a = shard.reshape(cfg.ni, cfg.iw, cfg.nkg, cfg.kpg, P)
        a = np.ascontiguousarray(a.transpose(0, 2, 4, 3, 1))
        a = a.reshape(cfg.ni, cfg.nkg, P, cfg.kpg * cfg.iw)
        in_maps.append({"adjt": a, "xt": xt, "w1": w1, "b1": b1d, "w2": w2})
    return in_maps


def finish_on_host(per_core_out, b2, W3, b3):
    """per_core_out: [n_cores, 2] local maxima -> [1,1,1] final output."""
    b2 = np.asarray(b2, dtype=np.float32)
    W3 = np.asarray(W3, dtype=np.float32)
    b3 = np.asarray(b3, dtype=np.float32)
    pooled = per_core_out.max(axis=0).astype(np.float32) + b2          # [2]
    out = pooled[None, None, :] @ W3.T + b3                            # [1,1,1]
    return out.astype(np.float32)


_NC_CACHE: dict = {}
LAST_RESULT = None  # BassKernelResults of the most recent run (for test.py)


def kernel(x, adj, W1, b1, W2, b2, W3, b3):
    cfg = Cfg()
    x = np.asarray(x)
    assert x.shape == (cfg.n, cfg.n_feat), x.shape
    if "nc" not in _NC_CACHE:
        _NC_CACHE["nc"] = build_nc(cfg)
    nc = _NC_CACHE["nc"]

    in_maps = shard_inputs(cfg, x, adj, W1, b1, W2)
    trace = os.environ.get("GCN_TRACE", "0") == "1"
    res = run_bass_kernel_spmd(
        nc, in_maps, core_ids=list(range(cfg.n_cores)), trace=trace)
    global LAST_RESULT
    LAST_RESULT = res
    per_core = np.stack(
        [np.asarray(r["out"][:, 0], dtype=np.float32) for r in res.results])
    return finish_on_host(per_core, b2, W3, b3)
